# revision 1
# baseline (speedup 1.0000x reference)
"""Trainium2 Bass kernel: GatedRecurrentCell.

Math (per batch b):
    pa = x @ Wa^T + ba ; pi = x @ Wi^T + bi
    a  = sigmoid(gate) * 3**(-sigmoid(pa))
       = exp(-ln3/2 * tanh((pa+ba)/2) + (ln(sigmoid(gate)) - ln3/2))
    c  = sqrt(1-a^2) * silu(pi + bi)
    h_t = a_t*h_{t-1} + c_t   (scan over time, h_{-1}=0);  out = h

Mapping: data-parallel over batch (8 cores, 1 batch each). On-chip layout is
channels-on-partitions / time-on-free-dim so the recurrence runs natively on
the DVE `tensor_tensor_scan` instruction. The host feeds pre-transposed
operand layouts (d-major x and W for the PE's contraction-on-partitions
matmul) and transposes the [I,S] per-core result back to [S,I] on the host.

The sigmoid for the decay gate is computed as a tanh so that it lives in the
same activation-table set as Exp (sigmoid/silu/exp/sqrt are all in different
sets; a set switch costs a 1.28us table load). ACT instruction order is
pinned with add_dep_helper so same-table phases run back-to-back.
"""

import functools
import os

import numpy as np

B, S, D, I = 8, 2048, 512, 2048
P = 128
NCORES = 8
LN3 = float(np.log(3.0))

# matmul input dtype: "f32r" (full-rate fp32 mode) or "f32" (4x slower, exact)
MM_MODE = os.environ.get("GRC_MM_MODE", "f32r")
IC_GROUP = int(os.environ.get("GRC_IC_GROUP", "2"))
# free-dim tile width for elementwise work (also the PSUM supertile width)
CW = int(os.environ.get("GRC_CW", "1024"))
# which engine runs the c = q*w muls: "gpsimd" or "vector"
CMUL_ENGINE = os.environ.get("GRC_CMUL", "gpsimd")
# every Nth channel-chunk's scan runs on gpsimd (0 = all on DVE)
SCAN_GP_MOD = int(os.environ.get("GRC_SCAN_GP", "0"))


def _build_nc(s, d, i, mm_mode=MM_MODE, ic_group=IC_GROUP, cw=CW,
              cmul_engine=CMUL_ENGINE, scan_gp_mod=SCAN_GP_MOD, silu=True):
    import concourse.bacc as bacc
    import concourse.mybir as mybir
    import concourse.tile as tile
    from concourse.tile import add_dep_helper

    F32 = mybir.dt.float32
    F32R = mybir.dt.float32r
    AF = mybir.ActivationFunctionType
    ALU = mybir.AluOpType

    nd = d // P          # contraction chunks
    ni = i // P          # channel chunks (partition groups)
    cw = min(cw, s)
    nh = s // cw         # elementwise chunks per channel row
    nmm = cw // 512      # matmuls (N=512) per psum supertile
    MMDT = F32R if mm_mode == "f32r" else F32

    nc = bacc.Bacc("TRN2", target_bir_lowering=False, debug=False,
                   num_devices=NCORES)

    xT_d = nc.dram_tensor("xT", [d, s], F32, kind="ExternalInput").ap()
    waT_d = nc.dram_tensor("WaT", [ni, P, d], F32, kind="ExternalInput").ap()
    wiT_d = nc.dram_tensor("WiT", [ni, P, d], F32, kind="ExternalInput").ap()
    ba_d = nc.dram_tensor("baT", [P, ni], F32, kind="ExternalInput").ap()
    bi_d = nc.dram_tensor("biT", [P, ni], F32, kind="ExternalInput").ap()
    g_d = nc.dram_tensor("gateT", [P, ni], F32, kind="ExternalInput").ap()
    out_d = nc.dram_tensor("out", [i, s], F32, kind="ExternalOutput").ap()

    with tile.TileContext(nc) as tc:
        from contextlib import ExitStack

        with ExitStack() as ctx:
            const_pool = ctx.enter_context(tc.tile_pool(name="const", bufs=1))
            xt_pool = ctx.enter_context(tc.tile_pool(name="xt", bufs=1))
            wst_pool = ctx.enter_context(tc.tile_pool(name="wstream", bufs=1))
            ps_pool = ctx.enter_context(
                tc.tile_pool(name="mmpsum", bufs=1, space="PSUM"))
            chunk_pool = ctx.enter_context(tc.tile_pool(name="chunks", bufs=1))
            row_pool = ctx.enter_context(tc.tile_pool(name="rows", bufs=1))

            nbuf_pw = 2 * ic_group * nh // 2 + 1   # phase-wide chunk pools

            # ---- per-channel vectors -------------------------------------
            ba_t = const_pool.tile([P, ni], F32, name="ba_t")
            nc.sync.dma_start(ba_t[:], ba_d[:])
            bi_t = const_pool.tile([P, ni], F32, name="bi_t")
            nc.sync.dma_start(bi_t[:], bi_d[:])
            g_t = const_pool.tile([P, ni], F32, name="g_t")
            nc.sync.dma_start(g_t[:], g_d[:])

            act_chain = []

            def act(out_ap, in_ap, func, **kw):
                inst = nc.scalar.activation(out_ap, in_ap, func, **kw)
                if act_chain:
                    add_dep_helper(inst.ins, act_chain[-1].ins, False,
                                   "act table phase order")
                act_chain.append(inst)
                return inst

            alpha_t = const_pool.tile([P, ni], F32, name="alpha_t")
            act(alpha_t[:], g_t[:], AF.Sigmoid)
            lna_t = const_pool.tile([P, ni], F32, name="lna_t")
            act(lna_t[:], alpha_t[:], AF.Ln)
            # exp-phase bias: ln(alpha) - ln3/2 ; tanh-phase bias: ba/2
            lnam_t = const_pool.tile([P, ni], F32, name="lnam_t")
            nc.vector.tensor_scalar_add(lnam_t[:], lna_t[:], -LN3 / 2.0)
            bah_t = const_pool.tile([P, ni], F32, name="bah_t")
            nc.vector.tensor_scalar_mul(bah_t[:], ba_t[:], 0.5)

            # ---- resident x^T tiles -------------------------------------
            xT_sb = []
            for k in range(nd):
                t_ = xt_pool.tile([P, s], MMDT, name=f"xT{k}", tag=f"xT{k}")
                xT_sb.append(t_)
            # column-chunked, k-interleaved loads so the first GEMM's
            # operands (all k, first columns) arrive as early as possible
            for h in range(nh):
                for k in range(nd):
                    nc.sync.dma_start(
                        xT_sb[k][:, h * cw:(h + 1) * cw],
                        xT_d[k * P:(k + 1) * P,
                             h * cw:(h + 1) * cw].bitcast(MMDT))

            def gemm(ps, w_sb, h):
                for m in range(nmm):
                    lo = h * cw + m * 512
                    for k in range(nd):
                        nc.tensor.matmul(
                            ps[:, m * 512:(m + 1) * 512],
                            w_sb[:, k * P:(k + 1) * P],
                            xT_sb[k][:, lo:lo + 512],
                            start=(k == 0), stop=(k == nd - 1))

            # wide grain for SBUF->SBUF elementwise stages
            ew = min(2 * cw, s)
            new = s // ew          # wide chunks per channel row

            # ---- main loop: groups of `ic_group` channel chunks ---------
            # Per group, a/wc/h live in ONE [P, len(ics)*s] buffer so the
            # recurrence runs as a single scan across all the group's
            # channels: a[channel_start] is zeroed, which exactly restarts
            # the recurrence (h0 = a0*0 + c0 never reads a0).
            groups = [list(range(g0, min(g0 + ic_group, ni)))
                      for g0 in range(0, ni, ic_group)]
            if ic_group > 1 and ni > 2:
                # split the final group into singletons: the kernel tail is
                # the last group's (c-mul -> scan -> DMA) chain, so keep it
                # short and run its muls on the faster DVE
                last = groups.pop()
                groups.extend([ic] for ic in last)
            for ics in groups:
                is_tail = len(ics) == 1
                gs = len(ics) * s      # group row length

                # stream weights + GEMMs (PE); all pi GEMMs for the group
                # first (the silu phase consumes them first), then all pa.
                pa_ps, pi_ps = {}, {}
                wa_sbs = {}
                for ic in ics:
                    wi_sb = wst_pool.tile([P, d], MMDT, name=f"wi{ic}",
                                          tag="wi", bufs=3)
                    nc.sync.dma_start(wi_sb[:], wiT_d[ic].bitcast(MMDT))
                    wa_sb = wst_pool.tile([P, d], MMDT, name=f"wa{ic}",
                                          tag="wa", bufs=3)
                    nc.sync.dma_start(wa_sb[:], waT_d[ic].bitcast(MMDT))
                    wa_sbs[ic] = wa_sb
                    for h in range(nh):
                        ps = ps_pool.tile([P, cw], F32, name=f"pi{ic}_{h}",
                                          tag="pi", bufs=2)
                        gemm(ps, wi_sb, h)
                        pi_ps[ic, h] = ps
                for ic in ics:
                    for h in range(nh):
                        ps = ps_pool.tile([P, cw], F32, name=f"pa{ic}_{h}",
                                          tag="pa", bufs=2)
                        gemm(ps, wa_sbs[ic], h)
                        pa_ps[ic, h] = ps

                wc_g = row_pool.tile([P, gs], F32, name=f"wc{ics[0]}",
                                     tag="wc", bufs=2)
                a_g = row_pool.tile([P, gs], F32, name=f"ag{ics[0]}",
                                    tag="ag", bufs=2)
                h_g = row_pool.tile([P, gs], F32, name=f"hg{ics[0]}",
                                    tag="hg", bufs=2)

                # ACT phase 1 [silu table]: w = silu(pi + bi) into wc -----
                for icg, ic in enumerate(ics):
                    for h in range(nh):
                        wt = wc_g[:, icg * s + h * cw: icg * s + (h + 1) * cw]
                        if silu:
                            act(wt, pi_ps[ic, h][:], AF.Silu,
                                bias=bi_t[:, ic:ic + 1])
                        else:
                            # sim-compatible fallback (Silu not in CoreSim)
                            sg = chunk_pool.tile(
                                [P, cw], F32, name=f"sg{ic}_{h}",
                                tag="sg", bufs=3)
                            act(sg[:], pi_ps[ic, h][:], AF.Sigmoid,
                                bias=bi_t[:, ic:ic + 1])
                            pib = chunk_pool.tile(
                                [P, cw], F32, name=f"pib{ic}_{h}",
                                tag="pib", bufs=3)
                            act(pib[:], pi_ps[ic, h][:], AF.Identity,
                                bias=bi_t[:, ic:ic + 1])
                            nc.vector.tensor_mul(wt, sg[:], pib[:])

                # ACT phase 2 [exp table]: t = tanh(pa/2 + ba/2) ----------
                s_t = {}
                for ic in ics:
                    for hw in range(new):
                        st = chunk_pool.tile([P, ew], F32, name=f"s{ic}_{hw}",
                                             tag="s", bufs=3)
                        for j in range(ew // cw):
                            act(st[:, j * cw:(j + 1) * cw],
                                pa_ps[ic, hw * (ew // cw) + j][:], AF.Tanh,
                                scale=0.5, bias=bah_t[:, ic:ic + 1])
                        s_t[ic, hw] = st

                # ACT phase 3 [exp table, no reload]:
                #   a = exp(-ln3/2 * t + (ln(alpha) - ln3/2))
                for icg, ic in enumerate(ics):
                    for hw in range(new):
                        act(a_g[:, icg * s + hw * ew: icg * s + (hw + 1) * ew],
                            s_t[ic, hw][:], AF.Exp,
                            scale=-LN3 / 2.0, bias=lnam_t[:, ic:ic + 1])
                # DVE: a2 = a*a (interleaves with ACT phases) -------------
                a2_t = {}
                for icg, ic in enumerate(ics):
                    for hw in range(new):
                        a2 = chunk_pool.tile([P, ew], F32,
                                             name=f"a2{ic}_{hw}",
                                             tag="s", bufs=3)
                        sl = a_g[:, icg * s + hw * ew: icg * s + (hw + 1) * ew]
                        nc.vector.tensor_mul(a2[:], sl, sl)
                        a2_t[ic, hw] = a2
                    if icg > 0:
                        # restart the recurrence at this channel boundary
                        # (a0 is never read by the scan: h0 = a0*0 + c0;
                        #  must happen AFTER a2 has consumed the real a0)
                        nc.gpsimd.memset(a_g[:, icg * s: icg * s + 1], 0.0)

                # ACT phase 4 [sqrt table]: q = sqrt(1 - a2);
                # then wc *= q in place (c = q*w), split DVE/gpsimd
                for icg, ic in enumerate(ics):
                    for hw in range(new):
                        q = chunk_pool.tile([P, ew], F32, name=f"q{ic}_{hw}",
                                            tag="q", bufs=3)
                        act(q[:], a2_t[ic, hw][:], AF.Sqrt,
                            scale=-1.0, bias=1.0)
                        wt = wc_g[:, icg * s + hw * ew:
                                  icg * s + (hw + 1) * ew]
                        eng = (nc.gpsimd
                               if cmul_engine == "gpsimd" and not is_tail
                               else nc.vector)
                        eng.tensor_mul(wt, q[:], wt)

                # one scan across the whole group's channels --------------
                nc.vector.tensor_tensor_scan(
                    h_g[:], a_g[:], wc_g[:], 0.0,
                    op0=ALU.mult, op1=ALU.add)
                for icg, ic in enumerate(ics):
                    nc.sync.dma_start(out_d[ic * P:(ic + 1) * P, :],
                                      h_g[:, icg * s:(icg + 1) * s])

    nc.compile()
    return nc


@functools.lru_cache(maxsize=2)
def _get_nc(s=S, d=D, i=I):
    return _build_nc(s, d, i)


LAST_RESULTS = None


def _prep_core_inputs(xb, WaT, WiT, baT, biT, gateT):
    return {"xT": np.ascontiguousarray(xb.T), "WaT": WaT, "WiT": WiT,
            "baT": baT, "biT": biT, "gateT": gateT}


def _prep_shared(Wa, ba, Wi, bi, gate, d, i):
    ni = i // P
    nd = d // P
    # WaT[ic, p, k*128+j] = Wa[ic*128+j, k*128+p]  (lhsT blocks, contiguous)
    WaT = np.ascontiguousarray(
        Wa.reshape(ni, P, nd, P).transpose(0, 3, 2, 1).reshape(ni, P, d))
    WiT = np.ascontiguousarray(
        Wi.reshape(ni, P, nd, P).transpose(0, 3, 2, 1).reshape(ni, P, d))
    baT = np.ascontiguousarray(ba.reshape(ni, P).T)
    biT = np.ascontiguousarray(bi.reshape(ni, P).T)
    gateT = np.ascontiguousarray(gate.reshape(ni, P).T)
    return WaT, WiT, baT, biT, gateT


def kernel(x, Wa, ba, Wi, bi, gate):
    global LAST_RESULTS
    from concourse.bass_utils import run_bass_kernel_spmd

    x = np.asarray(x, dtype=np.float32)
    b, s, d = x.shape
    i = Wa.shape[0]
    nc = _get_nc(s, d, i)

    WaT, WiT, baT, biT, gateT = _prep_shared(
        np.asarray(Wa, np.float32), np.asarray(ba, np.float32),
        np.asarray(Wi, np.float32), np.asarray(bi, np.float32),
        np.asarray(gate, np.float32), d, i)

    in_maps = [_prep_core_inputs(x[bb], WaT, WiT, baT, biT, gateT)
               for bb in range(b)]
    res = run_bass_kernel_spmd(nc, in_maps, list(range(b)))
    LAST_RESULTS = res
    out = np.stack([res.results[bb]["out"].T for bb in range(b)], axis=0)
    return np.ascontiguousarray(out, dtype=np.float32)



# revision 4
# speedup vs baseline: 1.0176x; 1.0176x over previous
"""Trainium2 Bass kernel: GatedRecurrentCell (v2).

Math (per batch b, channels on partitions, time on free dim):
    pa = x @ Wa^T (+ba) ; pi = x @ Wi^T (+bi)
    w  = silu(pi + bi)                      [ACT, silu table]
    t  = tanh(pa/2 + ba/2)                  [ACT, same table set as silu]
    a  = exp(-ln3/2 * t + (ln(sigmoid(g)) - ln3/2))   [ACT, exp table]
    a2 = a*a                                [DVE]
    q  = sqrt(1 - a2)                       [ACT, sqrt table]
    c  = q * w                              [GpSimd]
    h  = scan(h = a*h + c), h0 = 0          [DVE tensor_tensor_scan]

Mapping: data-parallel over batch (8 cores, 1 batch each). GEMM inputs in
bf16 (error enters before the sigmoid, so the sensitive q=sqrt(1-a^2)
chain is unaffected); t/a/a2/c stay fp32 (q has d(q)/q = 500*d(a)/a worst
case); w and q in bf16 (their rounding enters h un-amplified).

Structure: 16 channel chunks of 128, processed in cycles of C=4 chunks so
the ACT table loads (1.28us each) amortize: per cycle [silu+tanh x4]
(one table set) -> [exp x4] -> [sqrt x4] = 3 loads/cycle. Full-width
[128,2048] PSUM tiles (pi+pa = 8 banks); silu/tanh read PSUM directly.
alpha-derived constants are computed on the host.
"""

import functools
import os

import numpy as np

B, S, D, I = 8, 2048, 512, 2048
P = 128
NCORES = 8
LN3 = float(np.log(3.0))

# GEMM input dtype: "bf16" (default) or "f32r"
MM_DT = os.environ.get("GRC_MM_DT", "bf16")
# chunks per table-phase cycle
CYC = int(os.environ.get("GRC_C", "4"))
# engine for c = q*w: "gp" | "dve"
WC_ENGINE = os.environ.get("GRC_WC", "dve")
# engine for a2 = a*a: "dve" | "gp"
A2_ENGINE = os.environ.get("GRC_A2", "gp")
# dtype for w and q intermediates: "bf16" | "f32"
WQ_DT = os.environ.get("GRC_WQ_DT", "bf16")


def _build_nc(s, d, i, mm_dt=MM_DT, cyc=CYC, wc_engine=WC_ENGINE,
              a2_engine=A2_ENGINE, wq_dt=WQ_DT, silu=True):
    import concourse.bacc as bacc
    import concourse.mybir as mybir
    import concourse.tile as tile
    from concourse.tile import add_dep_helper

    F32 = mybir.dt.float32
    BF16 = mybir.dt.bfloat16
    AF = mybir.ActivationFunctionType
    ALU = mybir.AluOpType

    MMDT = BF16 if mm_dt == "bf16" else mybir.dt.float32r
    WQDT = BF16 if wq_dt == "bf16" else F32
    nd = d // P          # contraction chunks
    ni = i // P          # channel chunks
    cyc = min(cyc, ni)
    nmm = s // 512       # matmuls (N=512) per chunk-row GEMM

    nc = bacc.Bacc("TRN2", target_bir_lowering=False, debug=False,
                   num_devices=NCORES)

    xT_d = nc.dram_tensor("xT", [d, s], MMDT, kind="ExternalInput").ap()
    waT_d = nc.dram_tensor("WaT", [ni, P, d], MMDT, kind="ExternalInput").ap()
    wiT_d = nc.dram_tensor("WiT", [ni, P, d], MMDT, kind="ExternalInput").ap()
    bi_d = nc.dram_tensor("biT", [P, ni], F32, kind="ExternalInput").ap()
    bah_d = nc.dram_tensor("bahT", [P, ni], F32, kind="ExternalInput").ap()
    lnam_d = nc.dram_tensor("lnamT", [P, ni], F32, kind="ExternalInput").ap()
    out_d = nc.dram_tensor("out", [i, s], F32, kind="ExternalOutput").ap()

    with tile.TileContext(nc) as tc:
        from contextlib import ExitStack

        with ExitStack() as ctx:
            const_pool = ctx.enter_context(tc.tile_pool(name="const", bufs=1))
            xt_pool = ctx.enter_context(tc.tile_pool(name="xt", bufs=1))
            wst_pool = ctx.enter_context(tc.tile_pool(name="wstream", bufs=1))
            ps_pool = ctx.enter_context(
                tc.tile_pool(name="mmpsum", bufs=1, space="PSUM"))
            sb_pool = ctx.enter_context(tc.tile_pool(name="work", bufs=1))

            # ---- per-channel constant vectors --------------------------
            bi_t = const_pool.tile([P, ni], F32, name="bi_t")
            nc.sync.dma_start(bi_t[:], bi_d[:])
            bah_t = const_pool.tile([P, ni], F32, name="bah_t")
            nc.sync.dma_start(bah_t[:], bah_d[:])
            lnam_t = const_pool.tile([P, ni], F32, name="lnam_t")
            nc.sync.dma_start(lnam_t[:], lnam_d[:])

            act_chain = []

            def act(out_ap, in_ap, func, **kw):
                inst = nc.scalar.activation(out_ap, in_ap, func, **kw)
                if act_chain:
                    add_dep_helper(inst.ins, act_chain[-1].ins, False,
                                   "act table phase order")
                act_chain.append(inst)
                return inst

            # ---- resident x^T tiles ------------------------------------
            xT_sb = []
            for k in range(nd):
                xT_sb.append(xt_pool.tile([P, s], MMDT, name=f"xT{k}"))
            # column-chunked, k-interleaved loads: first GEMM's operands
            # arrive as early as possible
            xcw = min(512, s)
            for h in range(s // xcw):
                for k in range(nd):
                    nc.sync.dma_start(
                        xT_sb[k][:, h * xcw:(h + 1) * xcw],
                        xT_d[k * P:(k + 1) * P, h * xcw:(h + 1) * xcw])

            def gemm(ps, w_sb):
                for m in range(nmm):
                    for k in range(nd):
                        nc.tensor.matmul(
                            ps[:, m * 512:(m + 1) * 512],
                            w_sb[:, k * P:(k + 1) * P],
                            xT_sb[k][:, m * 512:(m + 1) * 512],
                            start=(k == 0), stop=(k == nd - 1))

            cycles = [list(range(c0, min(c0 + cyc, ni)))
                      for c0 in range(0, ni, cyc)]

            for ics in cycles:
                # ---- PE + phase 1 [silu table set: silu + tanh] --------
                w_t, t_t = {}, {}
                for ic in ics:
                    wi_sb = wst_pool.tile([P, d], MMDT, name=f"wi{ic}",
                                          tag="wi", bufs=3)
                    nc.sync.dma_start(wi_sb[:], wiT_d[ic])
                    wa_sb = wst_pool.tile([P, d], MMDT, name=f"wa{ic}",
                                          tag="wa", bufs=3)
                    nc.sync.dma_start(wa_sb[:], waT_d[ic])

                    pi_ps = ps_pool.tile([P, s], F32, name=f"pi{ic}",
                                         tag="pi", bufs=1)
                    gemm(pi_ps, wi_sb)
                    pa_ps = ps_pool.tile([P, s], F32, name=f"pa{ic}",
                                         tag="pa", bufs=1)
                    gemm(pa_ps, wa_sb)

                    wt = sb_pool.tile([P, s], WQDT, name=f"w{ic}", tag="w",
                                      bufs=cyc + 1)
                    if silu:
                        act(wt[:], pi_ps[:], AF.Silu, bias=bi_t[:, ic:ic + 1])
                    else:
                        # CoreSim fallback (Silu not interpreted)
                        sg = sb_pool.tile([P, s], F32, name=f"sg{ic}",
                                          tag="sg", bufs=2)
                        act(sg[:], pi_ps[:], AF.Sigmoid,
                            bias=bi_t[:, ic:ic + 1])
                        pib = sb_pool.tile([P, s], F32, name=f"pib{ic}",
                                           tag="pib", bufs=2)
                        act(pib[:], pi_ps[:], AF.Identity,
                            bias=bi_t[:, ic:ic + 1])
                        nc.vector.tensor_mul(wt[:], sg[:], pib[:])
                    w_t[ic] = wt

                    tt = sb_pool.tile([P, s], F32, name=f"t{ic}", tag="t",
                                      bufs=cyc)
                    act(tt[:], pa_ps[:], AF.Tanh, scale=0.5,
                        bias=bah_t[:, ic:ic + 1])
                    t_t[ic] = tt

                # ---- phase 2 [exp table set] ---------------------------
                a_t, a2_t = {}, {}
                for ic in ics:
                    at = sb_pool.tile([P, s], F32, name=f"a{ic}", tag="a",
                                      bufs=cyc + 1)
                    act(at[:], t_t[ic][:], AF.Exp, scale=-LN3 / 2.0,
                        bias=lnam_t[:, ic:ic + 1])
                    a_t[ic] = at
                    a2 = sb_pool.tile([P, s], F32, name=f"a2{ic}", tag="a2",
                                      bufs=cyc)
                    eng = nc.gpsimd if a2_engine == "gp" else nc.vector
                    eng.tensor_mul(a2[:], at[:], at[:])
                    a2_t[ic] = a2

                # ---- phase 3 [sqrt table set] + c-mul + scan + DMA -----
                q_t = {}
                for ic in ics:
                    qt = sb_pool.tile([P, s], WQDT, name=f"q{ic}", tag="q",
                                      bufs=3)
                    act(qt[:], a2_t[ic][:], AF.Sqrt, scale=-1.0, bias=1.0)
                    q_t[ic] = qt
                for ic in ics:
                    ct = sb_pool.tile([P, s], F32, name=f"c{ic}", tag="c",
                                      bufs=3)
                    eng = nc.gpsimd if wc_engine == "gp" else nc.vector
                    eng.tensor_mul(ct[:], q_t[ic][:], w_t[ic][:])
                    ht = sb_pool.tile([P, s], F32, name=f"h{ic}", tag="h",
                                      bufs=2)
                    nc.vector.tensor_tensor_scan(
                        ht[:], a_t[ic][:], ct[:], 0.0,
                        op0=ALU.mult, op1=ALU.add)
                    nc.sync.dma_start(out_d[ic * P:(ic + 1) * P, :], ht[:])

    nc.compile()
    return nc


@functools.lru_cache(maxsize=2)
def _get_nc(s=S, d=D, i=I):
    return _build_nc(s, d, i)


LAST_RESULTS = None


def _to_mm_dtype(arr):
    if MM_DT == "bf16":
        import ml_dtypes
        return arr.astype(ml_dtypes.bfloat16)
    return np.ascontiguousarray(arr)  # f32r: raw f32 bits


def _prep_core_inputs(xb, WaT, WiT, biT, bahT, lnamT):
    return {"xT": _to_mm_dtype(np.ascontiguousarray(xb.T)), "WaT": WaT,
            "WiT": WiT, "biT": biT, "bahT": bahT, "lnamT": lnamT}


def _prep_shared(Wa, ba, Wi, bi, gate, d, i):
    ni = i // P
    nd = d // P
    # WaT[ic, p, k*128+j] = Wa[ic*128+j, k*128+p]  (lhsT blocks, contiguous)
    WaT = _to_mm_dtype(
        Wa.reshape(ni, P, nd, P).transpose(0, 3, 2, 1).reshape(ni, P, d))
    WiT = _to_mm_dtype(
        Wi.reshape(ni, P, nd, P).transpose(0, 3, 2, 1).reshape(ni, P, d))
    biT = np.ascontiguousarray(bi.reshape(ni, P).T)
    bahT = np.ascontiguousarray((0.5 * ba).reshape(ni, P).T)
    # exp-phase bias: ln(sigmoid(gate)) - ln3/2, computed on host in f64
    g64 = gate.astype(np.float64)
    lnam = (-np.log1p(np.exp(-g64)) - LN3 / 2.0).astype(np.float32)
    lnamT = np.ascontiguousarray(lnam.reshape(ni, P).T)
    return WaT, WiT, biT, bahT, lnamT


def kernel(x, Wa, ba, Wi, bi, gate):
    global LAST_RESULTS
    from concourse.bass_utils import run_bass_kernel_spmd

    x = np.asarray(x, dtype=np.float32)
    b, s, d = x.shape
    i = Wa.shape[0]
    nc = _get_nc(s, d, i)

    WaT, WiT, biT, bahT, lnamT = _prep_shared(
        np.asarray(Wa, np.float32), np.asarray(ba, np.float32),
        np.asarray(Wi, np.float32), np.asarray(bi, np.float32),
        np.asarray(gate, np.float32), d, i)

    in_maps = [_prep_core_inputs(x[bb], WaT, WiT, biT, bahT, lnamT)
               for bb in range(b)]
    res = run_bass_kernel_spmd(nc, in_maps, list(range(b)))
    LAST_RESULTS = res
    out = np.stack([res.results[bb]["out"].T for bb in range(b)], axis=0)
    return np.ascontiguousarray(out, dtype=np.float32)


# revision 5
# speedup vs baseline: 1.0403x; 1.0222x over previous
"""Trainium2 Bass kernel: GatedRecurrentCell (v2.1, software-pipelined).

Math (per batch b, channels on partitions, time on free dim):
    w  = silu(pi + bi)                      [ACT, silu table]
    t  = tanh(pa/2 + ba/2)                  [ACT, same table set]
    a  = exp(-ln3/2 * t + (ln(sigmoid(g)) - ln3/2))   [ACT, exp table]
    a2 = a*a                                [GpSimd]
    q  = sqrt(1 - a2)                       [ACT, sqrt table]
    c  = q * w                              [DVE, all-bf16 2x]
    h  = scan(h = a*h + c), h0 = 0          [DVE tensor_tensor_scan]

Data-parallel over batch (8 cores, 1 batch each). GEMM inputs bf16 (the
GEMM rounding enters before the sigmoid, so the q = sqrt(1-a^2)
amplification ~a^2/q^2 does not see it); t/a/a2 stay fp32 (q would
amplify their rounding ~500x); w/q/c/h bf16 (enter h un-amplified).

Channel chunks (16 x 128) run in cycles of 3; ACT phases are
software-pipelined as P1(k+1) -> P2(k) -> P3(k): the PSUM-draining
phase P1 (silu+tanh, one table set) of the NEXT cycle executes between
the exp/sqrt phases of the current one, so the PE never waits a full
exp+sqrt window for PSUM (the v2.0 ping-pong). Table loads: 3/cycle.
"""

import functools
import os

import numpy as np

B, S, D, I = 8, 2048, 512, 2048
P = 128
NCORES = 8
LN3 = float(np.log(3.0))

MM_DT = os.environ.get("GRC_MM_DT", "bf16")      # "bf16" | "f32r"
CYC = int(os.environ.get("GRC_C", "3"))          # chunks per cycle
WC_ENGINE = os.environ.get("GRC_WC", "dve")      # c = q*w engine
A2_ENGINE = os.environ.get("GRC_A2", "gp")       # a2 = a*a engine
WQ_DT = os.environ.get("GRC_WQ_DT", "bf16")      # w/q dtype
CH_DT = os.environ.get("GRC_CH_DT", "bf16")      # c/h (scan in1/out) dtype


def _build_nc(s, d, i, mm_dt=MM_DT, cyc=CYC, wc_engine=WC_ENGINE,
              a2_engine=A2_ENGINE, wq_dt=WQ_DT, ch_dt=CH_DT, silu=True):
    import concourse.bacc as bacc
    import concourse.mybir as mybir
    import concourse.tile as tile
    from concourse.tile import add_dep_helper

    F32 = mybir.dt.float32
    BF16 = mybir.dt.bfloat16
    AF = mybir.ActivationFunctionType
    ALU = mybir.AluOpType

    MMDT = BF16 if mm_dt == "bf16" else mybir.dt.float32r
    WQDT = BF16 if wq_dt == "bf16" else F32
    CHDT = BF16 if ch_dt == "bf16" else F32
    nd = d // P
    ni = i // P
    cyc = min(cyc, ni)
    nmm = s // 512

    nc = bacc.Bacc("TRN2", target_bir_lowering=False, debug=False,
                   num_devices=NCORES)

    xT_d = nc.dram_tensor("xT", [d, s], MMDT, kind="ExternalInput").ap()
    waT_d = nc.dram_tensor("WaT", [ni, P, d], MMDT, kind="ExternalInput").ap()
    wiT_d = nc.dram_tensor("WiT", [ni, P, d], MMDT, kind="ExternalInput").ap()
    bi_d = nc.dram_tensor("biT", [P, ni], F32, kind="ExternalInput").ap()
    bah_d = nc.dram_tensor("bahT", [P, ni], F32, kind="ExternalInput").ap()
    lnam_d = nc.dram_tensor("lnamT", [P, ni], F32, kind="ExternalInput").ap()
    out_d = nc.dram_tensor("out", [i, s], CHDT, kind="ExternalOutput").ap()

    with tile.TileContext(nc) as tc:
        from contextlib import ExitStack

        with ExitStack() as ctx:
            const_pool = ctx.enter_context(tc.tile_pool(name="const", bufs=1))
            xt_pool = ctx.enter_context(tc.tile_pool(name="xt", bufs=1))
            wst_pool = ctx.enter_context(tc.tile_pool(name="wstream", bufs=1))
            ps_pool = ctx.enter_context(
                tc.tile_pool(name="mmpsum", bufs=1, space="PSUM"))
            sb_pool = ctx.enter_context(tc.tile_pool(name="work", bufs=1))

            bi_t = const_pool.tile([P, ni], F32, name="bi_t")
            nc.sync.dma_start(bi_t[:], bi_d[:])
            bah_t = const_pool.tile([P, ni], F32, name="bah_t")
            nc.sync.dma_start(bah_t[:], bah_d[:])
            lnam_t = const_pool.tile([P, ni], F32, name="lnam_t")
            nc.sync.dma_start(lnam_t[:], lnam_d[:])

            act_chain = []

            def act(out_ap, in_ap, func, **kw):
                inst = nc.scalar.activation(out_ap, in_ap, func, **kw)
                if act_chain:
                    add_dep_helper(inst.ins, act_chain[-1].ins, False,
                                   "act table phase order")
                act_chain.append(inst)
                return inst

            # ---- weight stream: first cycles' tiles load BEFORE x ------
            w_sb_cache = {}

            def load_w(ic):
                if ic in w_sb_cache:
                    return w_sb_cache[ic]
                wi_sb = wst_pool.tile([P, d], MMDT, name=f"wi{ic}",
                                      tag="wi", bufs=3)
                nc.sync.dma_start(wi_sb[:], wiT_d[ic])
                wa_sb = wst_pool.tile([P, d], MMDT, name=f"wa{ic}",
                                      tag="wa", bufs=3)
                nc.sync.dma_start(wa_sb[:], waT_d[ic])
                w_sb_cache[ic] = (wi_sb, wa_sb)
                return w_sb_cache[ic]

            for ic in range(min(2, ni)):
                load_w(ic)

            # ---- resident x^T tiles, k-interleaved column loads --------
            xT_sb = []
            for k in range(nd):
                xT_sb.append(xt_pool.tile([P, s], MMDT, name=f"xT{k}"))
            xcw = min(512, s)
            for h in range(s // xcw):
                for k in range(nd):
                    nc.sync.dma_start(
                        xT_sb[k][:, h * xcw:(h + 1) * xcw],
                        xT_d[k * P:(k + 1) * P, h * xcw:(h + 1) * xcw])

            def gemm(ps, w_sb):
                for m in range(nmm):
                    for k in range(nd):
                        nc.tensor.matmul(
                            ps[:, m * 512:(m + 1) * 512],
                            w_sb[:, k * P:(k + 1) * P],
                            xT_sb[k][:, m * 512:(m + 1) * 512],
                            start=(k == 0), stop=(k == nd - 1))

            cycles = [list(range(c0, min(c0 + cyc, ni)))
                      for c0 in range(0, ni, cyc)]

            w_t, t_t, a_t, a2_t = {}, {}, {}, {}

            def phase1(ics):
                for ic in ics:
                    wi_sb, wa_sb = load_w(ic)
                    pi_ps = ps_pool.tile([P, s], F32, name=f"pi{ic}",
                                         tag="pi", bufs=1)
                    gemm(pi_ps, wi_sb)
                    pa_ps = ps_pool.tile([P, s], F32, name=f"pa{ic}",
                                         tag="pa", bufs=1)
                    gemm(pa_ps, wa_sb)

                    wt = sb_pool.tile([P, s], WQDT, name=f"w{ic}", tag="w",
                                      bufs=2 * cyc + 1)
                    if silu:
                        act(wt[:], pi_ps[:], AF.Silu, bias=bi_t[:, ic:ic + 1])
                    else:  # CoreSim fallback (no Silu in interpreter)
                        sg = sb_pool.tile([P, s], F32, name=f"sg{ic}",
                                          tag="sg", bufs=2)
                        act(sg[:], pi_ps[:], AF.Sigmoid,
                            bias=bi_t[:, ic:ic + 1])
                        pib = sb_pool.tile([P, s], F32, name=f"pib{ic}",
                                           tag="pib", bufs=2)
                        act(pib[:], pi_ps[:], AF.Identity,
                            bias=bi_t[:, ic:ic + 1])
                        nc.vector.tensor_mul(wt[:], sg[:], pib[:])
                    w_t[ic] = wt

                    tt = sb_pool.tile([P, s], F32, name=f"t{ic}", tag="t",
                                      bufs=2 * cyc)
                    act(tt[:], pa_ps[:], AF.Tanh, scale=0.5,
                        bias=bah_t[:, ic:ic + 1])
                    t_t[ic] = tt

            def phase2(ics):
                for ic in ics:
                    at = sb_pool.tile([P, s], F32, name=f"a{ic}", tag="a",
                                      bufs=cyc + 2)
                    act(at[:], t_t[ic][:], AF.Exp, scale=-LN3 / 2.0,
                        bias=lnam_t[:, ic:ic + 1])
                    a_t[ic] = at
                    a2 = sb_pool.tile([P, s], F32, name=f"a2{ic}", tag="a2",
                                      bufs=cyc + 1)
                    eng = nc.gpsimd if a2_engine == "gp" else nc.vector
                    eng.tensor_mul(a2[:], at[:], at[:])
                    a2_t[ic] = a2

            def phase3(ics):
                for ic in ics:
                    qt = sb_pool.tile([P, s], WQDT, name=f"q{ic}", tag="q",
                                      bufs=3)
                    act(qt[:], a2_t[ic][:], AF.Sqrt, scale=-1.0, bias=1.0)
                    ct = sb_pool.tile([P, s], CHDT, name=f"c{ic}", tag="c",
                                      bufs=3)
                    eng = nc.gpsimd if wc_engine == "gp" else nc.vector
                    eng.tensor_mul(ct[:], qt[:], w_t[ic][:])
                    ht = sb_pool.tile([P, s], CHDT, name=f"h{ic}", tag="h",
                                      bufs=2)
                    nc.vector.tensor_tensor_scan(
                        ht[:], a_t[ic][:], ct[:], 0.0,
                        op0=ALU.mult, op1=ALU.add)
                    nc.sync.dma_start(out_d[ic * P:(ic + 1) * P, :], ht[:])

            # software pipeline: P1(k+1) runs before P2(k)/P3(k)
            phase1(cycles[0])
            for k in range(1, len(cycles)):
                phase1(cycles[k])
                phase2(cycles[k - 1])
                phase3(cycles[k - 1])
            phase2(cycles[-1])
            phase3(cycles[-1])

    nc.compile()
    return nc


@functools.lru_cache(maxsize=2)
def _get_nc(s=S, d=D, i=I):
    return _build_nc(s, d, i)


LAST_RESULTS = None


def _to_mm_dtype(arr):
    if MM_DT == "bf16":
        import ml_dtypes
        return arr.astype(ml_dtypes.bfloat16)
    return np.ascontiguousarray(arr)  # f32r: raw f32 bits


def _prep_core_inputs(xb, WaT, WiT, biT, bahT, lnamT):
    return {"xT": _to_mm_dtype(np.ascontiguousarray(xb.T)), "WaT": WaT,
            "WiT": WiT, "biT": biT, "bahT": bahT, "lnamT": lnamT}


def _prep_shared(Wa, ba, Wi, bi, gate, d, i):
    ni = i // P
    nd = d // P
    WaT = _to_mm_dtype(
        Wa.reshape(ni, P, nd, P).transpose(0, 3, 2, 1).reshape(ni, P, d))
    WiT = _to_mm_dtype(
        Wi.reshape(ni, P, nd, P).transpose(0, 3, 2, 1).reshape(ni, P, d))
    biT = np.ascontiguousarray(bi.reshape(ni, P).T)
    bahT = np.ascontiguousarray((0.5 * ba).reshape(ni, P).T)
    g64 = gate.astype(np.float64)
    lnam = (-np.log1p(np.exp(-g64)) - LN3 / 2.0).astype(np.float32)
    lnamT = np.ascontiguousarray(lnam.reshape(ni, P).T)
    return WaT, WiT, biT, bahT, lnamT


def kernel(x, Wa, ba, Wi, bi, gate):
    global LAST_RESULTS
    from concourse.bass_utils import run_bass_kernel_spmd

    x = np.asarray(x, dtype=np.float32)
    b, s, d = x.shape
    i = Wa.shape[0]
    nc = _get_nc(s, d, i)

    WaT, WiT, biT, bahT, lnamT = _prep_shared(
        np.asarray(Wa, np.float32), np.asarray(ba, np.float32),
        np.asarray(Wi, np.float32), np.asarray(bi, np.float32),
        np.asarray(gate, np.float32), d, i)

    in_maps = [_prep_core_inputs(x[bb], WaT, WiT, biT, bahT, lnamT)
               for bb in range(b)]
    res = run_bass_kernel_spmd(nc, in_maps, list(range(b)))
    LAST_RESULTS = res
    out = np.stack(
        [np.asarray(res.results[bb]["out"], dtype=np.float32).T
         for bb in range(b)], axis=0)
    return np.ascontiguousarray(out, dtype=np.float32)


# revision 6
# speedup vs baseline: 1.1534x; 1.1087x over previous
"""Trainium2 Bass kernel: GatedRecurrentCell (v2.1, software-pipelined).

Math (per batch b, channels on partitions, time on free dim):
    w  = silu(pi + bi)                      [ACT, silu table]
    t  = tanh(pa/2 + ba/2)                  [ACT, same table set]
    a  = exp(-ln3/2 * t + (ln(sigmoid(g)) - ln3/2))   [ACT, exp table]
    a2 = a*a                                [GpSimd]
    q  = sqrt(1 - a2)                       [ACT, sqrt table]
    c  = q * w                              [DVE, all-bf16 2x]
    h  = scan(h = a*h + c), h0 = 0          [DVE tensor_tensor_scan]

Data-parallel over batch (8 cores, 1 batch each). GEMM inputs bf16 (the
GEMM rounding enters before the sigmoid, so the q = sqrt(1-a^2)
amplification ~a^2/q^2 does not see it); t/a/a2 stay fp32 (q would
amplify their rounding ~500x); w/q/c/h bf16 (enter h un-amplified).

Channel chunks (16 x 128) run in cycles of 3; ACT phases are
software-pipelined as P1(k+1) -> P2(k) -> P3(k): the PSUM-draining
phase P1 (silu+tanh, one table set) of the NEXT cycle executes between
the exp/sqrt phases of the current one, so the PE never waits a full
exp+sqrt window for PSUM (the v2.0 ping-pong). Table loads: 3/cycle.
"""

import functools
import os

import numpy as np

B, S, D, I = 8, 2048, 512, 2048
P = 128
NCORES = 8
LN3 = float(np.log(3.0))

MM_DT = os.environ.get("GRC_MM_DT", "bf16")      # "bf16" | "f32r"
CYC = int(os.environ.get("GRC_C", "3"))          # chunks per cycle
WC_ENGINE = os.environ.get("GRC_WC", "dve")      # c = q*w engine
A2_ENGINE = os.environ.get("GRC_A2", "dve")       # a2 = a*a engine
WQ_DT = os.environ.get("GRC_WQ_DT", "bf16")      # w/q dtype
CH_DT = os.environ.get("GRC_CH_DT", "bf16")      # c/h (scan in1/out) dtype


def _build_nc(s, d, i, mm_dt=MM_DT, cyc=CYC, wc_engine=WC_ENGINE,
              a2_engine=A2_ENGINE, wq_dt=WQ_DT, ch_dt=CH_DT, silu=True):
    import concourse.bacc as bacc
    import concourse.mybir as mybir
    import concourse.tile as tile
    from concourse.tile import add_dep_helper

    F32 = mybir.dt.float32
    BF16 = mybir.dt.bfloat16
    AF = mybir.ActivationFunctionType
    ALU = mybir.AluOpType

    MMDT = BF16 if mm_dt == "bf16" else mybir.dt.float32r
    WQDT = BF16 if wq_dt == "bf16" else F32
    CHDT = BF16 if ch_dt == "bf16" else F32
    nd = d // P
    ni = i // P
    cyc = min(cyc, ni)
    nmm = s // 512

    nc = bacc.Bacc("TRN2", target_bir_lowering=False, debug=False,
                   num_devices=NCORES)

    xT_d = nc.dram_tensor("xT", [d, s], MMDT, kind="ExternalInput").ap()
    waT_d = nc.dram_tensor("WaT", [ni, P, d], MMDT, kind="ExternalInput").ap()
    wiT_d = nc.dram_tensor("WiT", [ni, P, d], MMDT, kind="ExternalInput").ap()
    bi_d = nc.dram_tensor("biT", [P, ni], F32, kind="ExternalInput").ap()
    bah_d = nc.dram_tensor("bahT", [P, ni], F32, kind="ExternalInput").ap()
    lnam_d = nc.dram_tensor("lnamT", [P, ni], F32, kind="ExternalInput").ap()
    out_d = nc.dram_tensor("out", [i, s], CHDT, kind="ExternalOutput").ap()

    with tile.TileContext(nc) as tc:
        from contextlib import ExitStack

        with ExitStack() as ctx:
            const_pool = ctx.enter_context(tc.tile_pool(name="const", bufs=1))
            xt_pool = ctx.enter_context(tc.tile_pool(name="xt", bufs=1))
            wst_pool = ctx.enter_context(tc.tile_pool(name="wstream", bufs=1))
            ps_pool = ctx.enter_context(
                tc.tile_pool(name="mmpsum", bufs=1, space="PSUM"))
            sb_pool = ctx.enter_context(tc.tile_pool(name="work", bufs=1))

            bi_t = const_pool.tile([P, ni], F32, name="bi_t")
            nc.sync.dma_start(bi_t[:], bi_d[:])
            bah_t = const_pool.tile([P, ni], F32, name="bah_t")
            nc.sync.dma_start(bah_t[:], bah_d[:])
            lnam_t = const_pool.tile([P, ni], F32, name="lnam_t")
            nc.sync.dma_start(lnam_t[:], lnam_d[:])

            act_chain = []

            def act(out_ap, in_ap, func, **kw):
                inst = nc.scalar.activation(out_ap, in_ap, func, **kw)
                if act_chain:
                    add_dep_helper(inst.ins, act_chain[-1].ins, False,
                                   "act table phase order")
                act_chain.append(inst)
                return inst

            # ---- weight stream: first cycles' tiles load BEFORE x ------
            w_sb_cache = {}

            def load_w(ic):
                if ic in w_sb_cache:
                    return w_sb_cache[ic]
                wi_sb = wst_pool.tile([P, d], MMDT, name=f"wi{ic}",
                                      tag="wi", bufs=3)
                nc.sync.dma_start(wi_sb[:], wiT_d[ic])
                wa_sb = wst_pool.tile([P, d], MMDT, name=f"wa{ic}",
                                      tag="wa", bufs=3)
                nc.sync.dma_start(wa_sb[:], waT_d[ic])
                w_sb_cache[ic] = (wi_sb, wa_sb)
                return w_sb_cache[ic]

            for ic in range(min(2, ni)):
                load_w(ic)

            # ---- resident x^T tiles, k-interleaved column loads --------
            xT_sb = []
            for k in range(nd):
                xT_sb.append(xt_pool.tile([P, s], MMDT, name=f"xT{k}"))
            xcw = min(512, s)
            for h in range(s // xcw):
                for k in range(nd):
                    nc.sync.dma_start(
                        xT_sb[k][:, h * xcw:(h + 1) * xcw],
                        xT_d[k * P:(k + 1) * P, h * xcw:(h + 1) * xcw])

            def gemm(ps, w_sb):
                for m in range(nmm):
                    for k in range(nd):
                        nc.tensor.matmul(
                            ps[:, m * 512:(m + 1) * 512],
                            w_sb[:, k * P:(k + 1) * P],
                            xT_sb[k][:, m * 512:(m + 1) * 512],
                            start=(k == 0), stop=(k == nd - 1))

            cycles = [list(range(c0, min(c0 + cyc, ni)))
                      for c0 in range(0, ni, cyc)]

            w_t, t_t, a_t, a2_t = {}, {}, {}, {}

            def phase1(ics):
                for ic in ics:
                    wi_sb, wa_sb = load_w(ic)
                    pi_ps = ps_pool.tile([P, s], F32, name=f"pi{ic}",
                                         tag="pi", bufs=1)
                    gemm(pi_ps, wi_sb)
                    pa_ps = ps_pool.tile([P, s], F32, name=f"pa{ic}",
                                         tag="pa", bufs=1)
                    gemm(pa_ps, wa_sb)

                    wt = sb_pool.tile([P, s], WQDT, name=f"w{ic}", tag="w",
                                      bufs=2 * cyc + 1)
                    if silu:
                        act(wt[:], pi_ps[:], AF.Silu, bias=bi_t[:, ic:ic + 1])
                    else:  # CoreSim fallback (no Silu in interpreter)
                        sg = sb_pool.tile([P, s], F32, name=f"sg{ic}",
                                          tag="sg", bufs=2)
                        act(sg[:], pi_ps[:], AF.Sigmoid,
                            bias=bi_t[:, ic:ic + 1])
                        pib = sb_pool.tile([P, s], F32, name=f"pib{ic}",
                                           tag="pib", bufs=2)
                        act(pib[:], pi_ps[:], AF.Identity,
                            bias=bi_t[:, ic:ic + 1])
                        nc.vector.tensor_mul(wt[:], sg[:], pib[:])
                    w_t[ic] = wt

                    tt = sb_pool.tile([P, s], F32, name=f"t{ic}", tag="t",
                                      bufs=2 * cyc)
                    act(tt[:], pa_ps[:], AF.Tanh, scale=0.5,
                        bias=bah_t[:, ic:ic + 1])
                    t_t[ic] = tt

            def phase2(ics):
                for ic in ics:
                    at = sb_pool.tile([P, s], F32, name=f"a{ic}", tag="a",
                                      bufs=cyc + 2)
                    act(at[:], t_t[ic][:], AF.Exp, scale=-LN3 / 2.0,
                        bias=lnam_t[:, ic:ic + 1])
                    a_t[ic] = at
                    a2 = sb_pool.tile([P, s], F32, name=f"a2{ic}", tag="a2",
                                      bufs=cyc + 1)
                    eng = nc.gpsimd if a2_engine == "gp" else nc.vector
                    eng.tensor_mul(a2[:], at[:], at[:])
                    a2_t[ic] = a2

            def phase3(ics):
                for ic in ics:
                    qt = sb_pool.tile([P, s], WQDT, name=f"q{ic}", tag="q",
                                      bufs=3)
                    act(qt[:], a2_t[ic][:], AF.Sqrt, scale=-1.0, bias=1.0)
                    ct = sb_pool.tile([P, s], CHDT, name=f"c{ic}", tag="c",
                                      bufs=3)
                    eng = nc.gpsimd if wc_engine == "gp" else nc.vector
                    eng.tensor_mul(ct[:], qt[:], w_t[ic][:])
                    ht = sb_pool.tile([P, s], CHDT, name=f"h{ic}", tag="h",
                                      bufs=2)
                    nc.vector.tensor_tensor_scan(
                        ht[:], a_t[ic][:], ct[:], 0.0,
                        op0=ALU.mult, op1=ALU.add)
                    nc.sync.dma_start(out_d[ic * P:(ic + 1) * P, :], ht[:])

            # software pipeline with interleaved excursions: cycle k's
            # P1 chunks alternate with cycle k-1's exp / sqrt bursts, so
            # each ACT excursion (~7us) matches the one-chunk PSUM
            # runway of the PE and neither engine waits a full window.
            from collections import deque
            pending = deque()
            for k, ics in enumerate(cycles):
                if k >= 1:
                    pending.append(("exp", cycles[k - 1]))
                    pending.append(("sqrt", cycles[k - 1]))
                for ic in ics:
                    phase1([ic])
                    if pending:
                        kind, pics = pending.popleft()
                        (phase2 if kind == "exp" else phase3)(pics)
            pending.append(("exp", cycles[-1]))
            pending.append(("sqrt", cycles[-1]))
            while pending:
                kind, pics = pending.popleft()
                (phase2 if kind == "exp" else phase3)(pics)

    nc.compile()
    return nc


@functools.lru_cache(maxsize=2)
def _get_nc(s=S, d=D, i=I):
    return _build_nc(s, d, i)


LAST_RESULTS = None


def _to_mm_dtype(arr):
    if MM_DT == "bf16":
        import ml_dtypes
        return arr.astype(ml_dtypes.bfloat16)
    return np.ascontiguousarray(arr)  # f32r: raw f32 bits


def _prep_core_inputs(xb, WaT, WiT, biT, bahT, lnamT):
    return {"xT": _to_mm_dtype(np.ascontiguousarray(xb.T)), "WaT": WaT,
            "WiT": WiT, "biT": biT, "bahT": bahT, "lnamT": lnamT}


def _prep_shared(Wa, ba, Wi, bi, gate, d, i):
    ni = i // P
    nd = d // P
    WaT = _to_mm_dtype(
        Wa.reshape(ni, P, nd, P).transpose(0, 3, 2, 1).reshape(ni, P, d))
    WiT = _to_mm_dtype(
        Wi.reshape(ni, P, nd, P).transpose(0, 3, 2, 1).reshape(ni, P, d))
    biT = np.ascontiguousarray(bi.reshape(ni, P).T)
    bahT = np.ascontiguousarray((0.5 * ba).reshape(ni, P).T)
    g64 = gate.astype(np.float64)
    lnam = (-np.log1p(np.exp(-g64)) - LN3 / 2.0).astype(np.float32)
    lnamT = np.ascontiguousarray(lnam.reshape(ni, P).T)
    return WaT, WiT, biT, bahT, lnamT


def kernel(x, Wa, ba, Wi, bi, gate):
    global LAST_RESULTS
    from concourse.bass_utils import run_bass_kernel_spmd

    x = np.asarray(x, dtype=np.float32)
    b, s, d = x.shape
    i = Wa.shape[0]
    nc = _get_nc(s, d, i)

    WaT, WiT, biT, bahT, lnamT = _prep_shared(
        np.asarray(Wa, np.float32), np.asarray(ba, np.float32),
        np.asarray(Wi, np.float32), np.asarray(bi, np.float32),
        np.asarray(gate, np.float32), d, i)

    in_maps = [_prep_core_inputs(x[bb], WaT, WiT, biT, bahT, lnamT)
               for bb in range(b)]
    res = run_bass_kernel_spmd(nc, in_maps, list(range(b)))
    LAST_RESULTS = res
    out = np.stack(
        [np.asarray(res.results[bb]["out"], dtype=np.float32).T
         for bb in range(b)], axis=0)
    return np.ascontiguousarray(out, dtype=np.float32)
